# revision 26
# baseline (speedup 1.0000x reference)
"""Multi-head scaled-dot-product attention on 8 Trainium2 NeuronCores.

Problem: x[4,2048,128], Wq/Wk/Wv[10,128,128] (torch Linear layout [e_out,d_in]),
Wo[128,1280], bo[128]  ->  out[4,2048,128]

Sharding: 8 cores = 4 batches x 2 head-groups (5 heads each). Each core
computes its batch's attention for its 5 heads plus the partial output
projection; the host sums the two half-head partials per batch, transposes,
and adds the bias.

Math folds (host side):
  W~_h = A * Wq_h^T @ Wk_h   with A = INV_SCALE * log2(e) * 1024, so the
       score matmul needs only raw x on the key side and produces scores
       pre-scaled for a 2^(t/1024) fp16-bits exponent evaluation.
  W2_h = WvT_h @ WoT_h       (V projection folded into output projection)

Per-core layout (all host-side pre-transposed; no on-chip transposes):
  xT   [d=128, n=2048]  = x[b].T       (scores lhsT AND qt~ projection rhs)
  xn   chunk-major natural x           (OT lhsT)
  qt~_h [f, n] = wt_h.T @ xT           (computed on HOST, DMA'd per block)
  ST   [m-chunk, nb]    = xT_chunk.T @ qt~_slice   (keys on partitions)
  PT   = 2^(ST/1024):
       - 7 pairs on ScalarE: ACTIVATE Exp with scale=ln2/1024 (exact);
         the 7-ACT chain is the pace-setter (~7.7us per (nb,h))
       - pair 4 on VectorE: Schraudolph bit-trick int16(ST + B) bitcast
         fp16, one chunk at a time through the dn PSUM bank so it never
         occupies a score slot (keeps the ACT chain stall-free)
  den  : pairwise DVE tree fold of the 8 PT pair tiles -> u[128,2,512],
       2 ones-matmuls (contracting keys) -> dn_ps, reciprocal on DVE.
       The den matmuls + reciprocal run in the NEXT head's chunk stream
       (cp==3) so the PE queue head never blocks on them.
  OT_h [e, nb] += xn_chunk.T @ PT_chunk   (emitted three pairs behind the
       exps; the last three pairs' OT carries into the next head's cp0-2)
  outT [dout, nb] += w2_h.T @ (OT_h * recip(den))  (next head's cp==5)
"""

from contextlib import ExitStack

import numpy as np

import concourse.tile as tile
from concourse import bacc, mybir
from concourse.bass import ds, ts
from concourse.bass_utils import run_bass_kernel_spmd

B, N, D, H = 4, 2048, 128, 10
HL = H // 2  # heads per core
NCHUNK = N // 128  # 16 key chunks
NPAIR = NCHUNK // 2  # 8 chunk pairs
NBLK = N // 512  # 4 query blocks
INV_SCALE = float(1.0 / (128.0**0.5 + 1e-8))
A_FOLD = INV_SCALE * (1.0 / float(np.log(2.0))) * 1024.0  # folded into W~
EXP_SCALE = float(np.log(2.0) / 1024.0)  # ScalarE: exp(EXP_SCALE*ST)=2^(ST/1024)
SCH_C = 59.37
SCH_BIAS = float(15 * 1024 - SCH_C)  # VectorE: fp16 bits = round(ST + BIAS)
SCH_PAIR = 4  # chunk pair whose exp runs on VectorE (Schraudolph)
f32 = mybir.dt.float32

PROFILE = False
LAST_RESULTS = None

_built = None


def _emit(tc, xT, xn, qtd, w2, ones_dram, outT):
    nc = tc.nc
    Exp = mybir.ActivationFunctionType.Exp
    Add = mybir.AluOpType.add
    fp16 = mybir.dt.float16
    i16 = mybir.dt.int16

    ctx = ExitStack()
    consts = ctx.enter_context(tc.tile_pool(name="consts", bufs=1))
    proj = ctx.enter_context(tc.tile_pool(name="proj", bufs=1))
    ps = ctx.enter_context(tc.tile_pool(name="ps", bufs=2, space="PSUM"))
    otps = ctx.enter_context(tc.tile_pool(name="otps", bufs=2, space="PSUM"))
    dnps = ctx.enter_context(tc.tile_pool(name="dnps", bufs=1, space="PSUM"))
    outps = ctx.enter_context(tc.tile_pool(name="outps", bufs=1, space="PSUM"))
    ptp = ctx.enter_context(tc.tile_pool(name="ptp", bufs=8))
    work = ctx.enter_context(tc.tile_pool(name="work", bufs=2))

    ones_mat = consts.tile([128, 128], fp16)
    xT_sb = consts.tile([D, N], fp16)
    xn_sb = consts.tile([D, N], fp16)  # chunk-major natural x: [p, c*128+d]
    w2_sb = consts.tile([D, HL * D], fp16)
    qt = consts.tile([D, NBLK, HL * 512], fp16)  # [d, nb, h*512+q]
    warm = consts.tile([128, 16], fp16)
    # critical first tiles split across queues (descriptor-latency bound);
    # the Scalar queue's DMA issues go ahead of the ACT-table preload
    nc.sync.dma_start(qt[0:48, 0, ds(0, 512)], qtd[0:48, 0, 0:512])
    nc.scalar.dma_start(qt[48:96, 0, ds(0, 512)], qtd[48:96, 0, 0:512])
    nc.gpsimd.dma_start(qt[96:128, 0, ds(0, 512)], qtd[96:128, 0, 0:512])
    nc.sync.dma_start(xT_sb[0:64, 0:512], xT[0:64, 0:512])
    nc.scalar.dma_start(xT_sb[64:128, 0:512], xT[64:128, 0:512])
    nc.sync.dma_start(xT_sb[:, 512:], xT[:, 512:])
    nc.gpsimd.dma_start(xn_sb[:], xn)
    nc.scalar.dma_start(qt[:, 0, ds(512, HL * 512 - 512)], qtd[:, 0, 512:])
    nc.sync.dma_start(qt[:, 1:NBLK, :], qtd[:, 1:NBLK, :])
    nc.gpsimd.dma_start(ones_mat[:], ones_dram)
    nc.gpsimd.dma_start(w2_sb[:], w2)
    # preload the exp ACT table while the DMAs stream (scale=0 -> exp(0))
    nc.vector.memset(warm[:], 0.0)
    nc.scalar.activation(warm[:], warm[:], Exp, scale=0.0)

    # ---- attention (software-pipelined emission) ----
    pend = None  # previous head's epilogue state

    def emit_den(st):
        # ones-matmuls contracting the folded accumulator's keys, then the
        # reciprocal broadcast. The final head skips the last fold level and
        # contracts v0/v1 with 4 matmuls to shorten the tail chain.
        dn_ps = dnps.tile([128, 512], f32, tag="dn")
        srcs = (
            [st["u"][:, j] for j in range(2)]
            if st["u"] is not None
            else [v[:, j] for v in st["v"] for j in range(2)]
        )
        for j, s in enumerate(srcs):
            nc.tensor.matmul(
                dn_ps[:],
                ones_mat[:],
                s,
                start=(j == 0),
                stop=(j == len(srcs) - 1),
            )
        bc = work.tile([128, 512], f32, tag="bc")
        nc.vector.reciprocal_approx_fast(out=bc[:], in_=dn_ps[:])
        st["bc"] = bc

    def emit_finish(st):
        otn = work.tile([128, 512], fp16, tag="otn")
        nc.vector.tensor_mul(otn[:], st["ot_ps"][:], st["bc"][:])
        nc.tensor.matmul(
            st["outp"][:],
            w2_sb[:, ts(st["h"], D)],
            otn[:],
            start=(st["h"] == 0),
            stop=(st["h"] == HL - 1),
        )
        if st["h"] == HL - 1:
            osb = work.tile([128, 512], f32, tag="osb")
            if st["nb"] == NBLK - 1:
                # final output block: evac on the (now idle) ScalarE and
                # split the DMA across two queues to halve its latency
                nc.scalar.copy(osb[:], st["outp"][:])
                nc.sync.dma_start(outT[0:48, ts(st["nb"], 512)], osb[0:48, :])
                nc.scalar.dma_start(
                    outT[48:96, ts(st["nb"], 512)], osb[48:96, :]
                )
                nc.gpsimd.dma_start(
                    outT[96:128, ts(st["nb"], 512)], osb[96:128, :]
                )
            else:
                nc.vector.tensor_copy(osb[:], st["outp"][:])
                nc.sync.dma_start(outT[:, ts(st["nb"], 512)], osb[:])

    for nb in range(NBLK):
        outp = outps.tile([128, 512], f32)
        for h in range(HL):
            ot_ps = otps.tile([128, 512], f32)
            pairs = {}
            wtl = {}

            def emit_ot(cp, pairs=pairs, ot_ps=ot_ps):
                pp = pairs[cp]
                for j in range(2):
                    cc = 2 * cp + j
                    nc.tensor.matmul(
                        ot_ps[:],
                        xn_sb[:, ts(cc, 128)],
                        pp[:, j],
                        start=(cc == 0),
                        stop=(cc == NCHUNK - 1),
                    )

            def sch_chunk(j):
                # the Schraudolph pair routes through the dn bank one chunk
                # at a time so it never occupies a score slot: the 7 ACT
                # pairs then recycle slots two pairs ahead, stall-free
                s1 = dnps.tile([128, 512], f32, tag="dn", name=f"sch{j}")
                nc.tensor.matmul(
                    s1[:],
                    xT_sb[:, ds((2 * SCH_PAIR + j) * 128, 128)],
                    qt[:, nb, ds(h * 512, 512)],
                    start=True,
                    stop=True,
                )
                nc.vector.tensor_scalar(
                    pairs[SCH_PAIR][:, j].bitcast(i16), s1[:], SCH_BIAS, None, Add
                )

            for cp in range(NPAIR):
                if cp == SCH_PAIR:
                    pass  # exp'd via sch_chunk at cp-1/cp, OT at cp+2
                else:
                    stp = ps.tile([128, 2, 512], f32, tag="st")
                    for j in range(2):
                        nc.tensor.matmul(
                            stp[:, j],
                            xT_sb[:, ds((2 * cp + j) * 128, 128)],
                            qt[:, nb, ds(h * 512, 512)],
                            start=True,
                            stop=True,
                        )
                    p = ptp.tile([128, 2, 512], fp16, tag="pt")
                    nc.scalar.activation(p[:], stp[:], Exp, scale=EXP_SCALE)
                    pairs[cp] = p
                if cp == SCH_PAIR + 1:
                    pairs[SCH_PAIR] = ptp.tile(
                        [128, 2, 512], fp16, tag="pt", name="psch"
                    )
                    sch_chunk(0)
                elif cp == SCH_PAIR + 2:
                    sch_chunk(1)
                # previous head's epilogue (and its two carried-over OT
                # pairs) ride this head's stream, queued ahead of the fold
                # adds so they are never delayed by them
                if pend is not None:
                    if cp == 0:
                        pend["emit_ot"](NPAIR - 3)
                    elif cp == 1:
                        pend["emit_ot"](NPAIR - 2)
                    elif cp == 2:
                        pend["emit_ot"](NPAIR - 1)
                    elif cp == 3:
                        emit_den(pend)
                    elif cp == 5:
                        emit_finish(pend)
                        pend = None
                # denominator tree fold on DVE, emitted two pairs behind the
                # exps so the Schraudolph op is never queued behind a fold
                def emit_fold(i):
                    wtl[i] = work.tile(
                        [128, 2, 512], fp16, tag=f"w{i}", name=f"w{i}"
                    )
                    nc.vector.tensor_add(
                        wtl[i][:], pairs[2 * i][:], pairs[2 * i + 1][:]
                    )

                if cp in (3, 5, 7):
                    emit_fold((cp - 3) // 2)
                    if cp == 5:
                        wtl["v0"] = work.tile(
                            [128, 2, 512], fp16, tag="v0", name="v0"
                        )
                        nc.vector.tensor_add(
                            wtl["v0"][:], wtl[0][:], wtl[1][:]
                        )
                # two-pair-deep pipelining for the PV accumulation; the
                # Schraudolph pair's OT is deferred two more slots so its
                # second chunk's exp has completed
                if cp >= 3:
                    emit_ot(cp - 3)
            if nb == NBLK - 1 and h == HL - 1:
                emit_ot(NPAIR - 3)
                emit_ot(NPAIR - 2)
                emit_ot(NPAIR - 1)
            emit_fold(3)
            wtl["v1"] = work.tile([128, 2, 512], fp16, tag="v1", name="v1")
            nc.vector.tensor_add(wtl["v1"][:], wtl[2][:], wtl[3][:])
            if nb == NBLK - 1 and h == HL - 1:
                u = None  # final head: den contracts v0/v1 directly
            else:
                u = work.tile([128, 2, 512], fp16, tag="u")
                nc.vector.tensor_add(u[:], wtl["v0"][:], wtl["v1"][:])
            pend = {
                "u": u,
                "v": (wtl["v0"], wtl["v1"]),
                "ot_ps": ot_ps,
                "emit_ot": emit_ot,
                "outp": outp,
                "h": h,
                "nb": nb,
            }
    emit_den(pend)
    emit_finish(pend)
    pend = None
    ctx.close()


def _build():
    fp16 = mybir.dt.float16
    nc = bacc.Bacc("TRN2", target_bir_lowering=False, debug=False)
    xT = nc.dram_tensor("xT", [D, N], fp16, kind="ExternalInput").ap()
    xn = nc.dram_tensor("xn", [D, N], fp16, kind="ExternalInput").ap()  # chunk-major
    qtd = nc.dram_tensor("qtd", [D, NBLK, HL * 512], fp16, kind="ExternalInput").ap()
    w2 = nc.dram_tensor("w2", [D, HL * D], fp16, kind="ExternalInput").ap()
    ones_dram = nc.dram_tensor("ones", [D, D], fp16, kind="ExternalInput").ap()
    outT = nc.dram_tensor("outT", [D, N], f32, kind="ExternalOutput").ap()
    with tile.TileContext(nc) as tc:
        with nc.allow_low_precision(reason="fp16 matmul operands"):
            _emit(tc, xT, xn, qtd, w2, ones_dram, outT)
    nc.compile()
    return nc


def kernel(x, Wq, Wk, Wv, Wo, bo):
    global _built, LAST_RESULTS
    x = np.asarray(x, dtype=np.float32)
    Wq = np.asarray(Wq, dtype=np.float32)
    Wk = np.asarray(Wk, dtype=np.float32)
    Wv = np.asarray(Wv, dtype=np.float32)
    Wo = np.asarray(Wo, dtype=np.float32)
    bo = np.asarray(bo, dtype=np.float32)

    if _built is None:
        _built = _build()
    nc = _built

    # W~_h = A * Wq_h^T @ Wk_h  (both [e_out, d_in] torch layout), then
    # the q-side projection qt~[h, f, n] = (x W~_h)^T on the host
    Wt = A_FOLD * np.einsum("hed,hef->hdf", Wq, Wk)
    QT = np.empty((B, H, D, N), dtype=np.float16)
    for b in range(B):
        for h in range(H):
            QT[b, h] = (x[b] @ Wt[h]).T.astype(np.float16)
    # per-core qt layout [d, nb, h*512+q]
    QTD = [
        np.ascontiguousarray(
            QT[b, g * HL : (g + 1) * HL]
            .reshape(HL, D, NBLK, 512)
            .transpose(1, 2, 0, 3)
            .reshape(D, NBLK, HL * 512)
        )
        for b in range(B)
        for g in range(2)
    ]
    # chunk-major xn: [p, c*128+d] = x[c*128+p, d]
    XNP = [
        np.ascontiguousarray(
            x[b].astype(np.float16).reshape(NCHUNK, 128, D).transpose(1, 0, 2).reshape(128, N)
        )
        for b in range(B)
    ]
    # fold the V projection into the output projection: W2_h = WvT_h @ WoT_h
    W2 = np.einsum(
        "hde,heo->hdo", Wv.transpose(0, 2, 1), Wo.T.reshape(H, D, D)
    ).astype(np.float16)
    W2C = [
        np.ascontiguousarray(
            W2[g * HL : (g + 1) * HL].transpose(1, 0, 2).reshape(D, HL * D)
        )
        for g in range(2)
    ]

    in_maps = []
    for c in range(8):
        b, g = divmod(c, 2)
        hsl = slice(g * HL, g * HL + HL)
        in_maps.append(
            {
                "xT": np.ascontiguousarray(x[b].T.astype(np.float16)),
                "xn": XNP[b],
                "qtd": QTD[c],
                "w2": W2C[g],
                "ones": np.ones((D, D), dtype=np.float16),
            }
        )

    res = run_bass_kernel_spmd(
        nc, in_maps, core_ids=list(range(8)), trace=PROFILE
    )
    LAST_RESULTS = res

    out = np.empty((B, N, D), dtype=np.float32)
    for b in range(B):
        oT = res.results[2 * b]["outT"] + res.results[2 * b + 1]["outT"]
        out[b] = oT.T
    out += bo
    return out
